# revision 40
# baseline (speedup 1.0000x reference)
"""Trainium2 Bass kernel for nn_Bottleneck_1w1a (binarized ResNet bottleneck).

Math notes (why this is exact):
  - The reference binarizes weights as sign(standardize(w)); standardization
    divides by a positive scalar and the tau-clip is identity (tau=1 -> +inf),
    so bw = sign(w - mean(w, per out-filter)).
  - Activations are binarized as sign(a), and hardtanh(v) has the sign of v,
    so the input to conv_{k+1} is sign(bn_k(conv_k * alpha_k)).  With
    k = alpha*gamma/sqrt(1+eps) >= 0, sign(c*k + b) for the *integer* conv
    output c is a threshold comparison c >= T.  T is precomputed on host by
    evaluating the reference's exact f32 expression on every achievable
    integer c, so the device does Sign(c + bias) with bias = -(T - step/2),
    which is exact in f32.
  - All matmul operands are exactly +-1 (or 0 from padding) which fp8e4m3
    represents exactly; PSUM accumulates in f32, so conv outputs are exact
    integers.  fp8 DoubleRow mode (2 contraction subtiles/instr) is lossless.
  - conv2 (3x3, pad 1) is done as 9 shifted accumulating matmuls over a
    zero-padded 30x30 per-image layout; the pad garbage columns are computed
    and discarded via strided access patterns.

Sharding: data-parallel over batch, 4 images per core, weights replicated.
"""

import os
import sys

import numpy as np

for _p in ("/opt/trn_rl_repo", "/root/.axon_site/_ro/pypackages"):
    if _p not in sys.path:
        sys.path.insert(0, _p)

from concourse import bacc, bass, mybir  # noqa: E402
from concourse.tile import TileContext  # noqa: E402
from concourse.bass_utils import run_bass_kernel_spmd  # noqa: E402

F8 = mybir.dt.float8e4
F32 = mybir.dt.float32
BF16 = mybir.dt.bfloat16
NP_F8 = mybir.dt.np(F8)
NP_BF16 = mybir.dt.np(BF16)
AF = mybir.ActivationFunctionType
DR = mybir.MatmulPerfMode.DoubleRow

N, CIN, H, W = 32, 1024, 28, 28
NCORES = 8
NIMG = N // NCORES          # 4 images per core
HW = H * W                  # 784
PW = W + 2                  # 30 padded width
PN = PW * (H + 2)           # 900 padded flat size
EPS = 1e-5

# psum column offset for each half-image chunk (each matmul's out must sit in
# one 2KB PSUM bank)
PSOFF = (0, 512)
# conv2 free-dim chunks over padded flat coords j (interior j in [31,869)):
# (start_j, width).  Chunk A covers rows 1..14, chunk B rows 15..28; widths
# chosen so every tap read j + (dy-1)*30 + (dx-1) stays inside [0, 900).
# The columns with j % 30 in {0, 29} are pad garbage, discarded by the
# strided interior view in the z2 activation.
CHUNKS = ((31, 420), (451, 418))


def _register_caac():
    """Register a fused DVE op: out = clip((in0*s0 + s1) + in1, -1, 1).
    This is the whole bn3 + residual + hardtanh tail in one DVE pass.
    Runtime registration via the documented per-NEFF DVE table mechanism;
    the pinned uop hashes are computed with the same lowering the golden
    test uses."""
    import concourse.dve_ops as DO
    from concourse.dve_spec import Spec, Src0, Src1, C0, C1, Zero, One, maxx, minn, lower
    from concourse.dve_uop import DveOpSpec

    name = "CLAMP_AFFINE_ADD_ANT"
    for op in DO.OPS:
        if op.name == name:
            return op

    def ref(in0, in1, s0, s1, imm2):
        p = in0.shape[0]
        a = in0.astype(np.float32).reshape(p, -1)
        b = np.asarray(in1, dtype=np.float32).reshape(p, -1)
        out = np.clip((a * s0 + s1) + b, -1.0, 1.0).astype(np.float32)
        return out

    spec = Spec(
        body=minn(maxx((Src0 * C0 + C1) + Src1, Zero - One), One),
        reference=ref,
    )
    shas = {
        v: DveOpSpec(name=name, opcode=0, uops=lower(spec, ver=v), rd1_en=True).sha(v)
        for v in ("v3", "v4")
    }
    op = DO.DveOp(name, spec, subdim=False, uops_sha=shas)
    DO.OPS.append(op)
    DO._SUB_OPCODE_FOR_NAME[name] = DO._CUSTOM_DVE_ROW_BASE + len(DO.OPS) - 1
    DO.CUSTOM_DVE_SPECS[name] = spec
    return op


def _dedup_ldweights(nc):
    """Remove LDWEIGHTS instructions that reload the stationary weights the
    PE already holds (Tile lowering emits one per matmul; consecutive
    matmuls that share lhsT only need the first).  Between a kept load and
    a dependent matmul only Matmult instructions occur, which do not
    disturb the stationary array.  Any sync info on a removed load is
    transplanted to the next PE instruction so semaphore counts are
    preserved."""
    import json

    def ldw_key(inst):
        j = json.loads(mybir.instruction_to_pretty_json_string(inst))
        return json.dumps(
            (j.get("ins"), j.get("perf_mode"), j.get("is_transpose")), sort_keys=True
        )

    removed = 0
    for blk in nc.m.functions[0].blocks:
        insts = blk.instructions
        last_key = None
        to_remove = []
        pending_sync = None
        for inst in insts:
            if inst.engine != mybir.EngineType.PE:
                continue
            if pending_sync is not None:
                si = inst.sync_info
                if si is None:
                    inst.sync_info = pending_sync
                else:
                    si.on_wait.extend(pending_sync.on_wait)
                    si.on_update.extend(pending_sync.on_update)
                pending_sync = None
            if inst.opcode == "Ldweights":
                k = ldw_key(inst)
                if k == last_key:
                    if inst.sync_info is not None and (
                        inst.sync_info.on_wait or inst.sync_info.on_update
                    ):
                        pending_sync = inst.sync_info
                    to_remove.append(inst)
                last_key = k
            elif inst.opcode == "Matmult":
                pass
            else:
                last_key = None
        assert pending_sync is None
        for inst in to_remove:
            insts.remove(inst)
            removed += 1
    return removed


def _build_nc():
    caac = _register_caac()
    nc = bacc.Bacc()
    x_d = nc.declare_dram_parameter("x", [128, NIMG, 8, HW], BF16, isOutput=False)
    w1_d = nc.declare_dram_parameter("w1t", [128, 8, 256], F8, isOutput=False)
    w2_d = nc.declare_dram_parameter("w2t", [128, 2, 9, 256], F8, isOutput=False)
    w3_d = nc.declare_dram_parameter("w3t", [128, 2, 1024], F8, isOutput=False)
    b1_d = nc.declare_dram_parameter("b1t", [128, 2], F32, isOutput=False)
    b2_d = nc.declare_dram_parameter("b2t", [128, 2], F32, isOutput=False)
    k3_d = nc.declare_dram_parameter("k3t", [128, 8], F32, isOutput=False)
    b3_d = nc.declare_dram_parameter("b3t", [128, 8], F32, isOutput=False)
    o_d = nc.declare_dram_parameter("out", [128, NIMG, 8, HW], BF16, isOutput=True)

    with TileContext(nc) as tc:
        with (
            tc.tile_pool(name="const", bufs=1) as cp,
            tc.tile_pool(name="xp", bufs=4) as xp,
            tc.tile_pool(name="t1p", bufs=2) as t1p,
            tc.tile_pool(name="z1p", bufs=2) as z1p,
            tc.tile_pool(name="z2p", bufs=2) as z2p,
            tc.tile_pool(name="op", bufs=2) as op,
            tc.tile_pool(name="c1p", bufs=1, space="PSUM") as c1p,
            tc.tile_pool(name="c2p", bufs=1, space="PSUM") as c2p,
            tc.tile_pool(name="c3p", bufs=2, space="PSUM") as c3p,
        ):
            w1s = cp.tile([128, 8, 256], F8)
            w2s = cp.tile([128, 2, 9, 256], F8)
            w3s = cp.tile([128, 2, 1024], F8)
            b1s = cp.tile([128, 2], F32)
            b2s = cp.tile([128, 2], F32)
            k3s = cp.tile([128, 8], F32)
            b3s = cp.tile([128, 8], F32)
            # Hand-pipelined schedule.  Stage order per iteration i:
            #   sign(i+1)  [DVE+Pool]  -> conv2(i) [PE] -> load_x(i+2) [SP]
            #   -> conv1(i+1) [PE, fills the z2(i) ACT gap]
            #   -> conv3(i)+tail(i) [PE/DVE/SP, fills the z1(i+1) ACT gap]
            xts, t1s, z1s, z2s = [], [], [], []

            def load_x(i):
                xt = xp.tile([128, 8, HW], BF16, tag="xt")
                for half in range(2):
                    nc.sync.dma_start(
                        out=xt[:, 4 * half : 4 * half + 4],
                        in_=x_d[:, i, 4 * half : 4 * half + 4],
                    )
                xts.append(xt)

            def sign_chunk_gpsimd(t1, xt, q):
                # sign(x) in fp8e4m3 is just (high_byte & 0x80) | 0x38, so
                # gpsimd can binarize from the bf16 sign bit with one
                # tensor_scalar, relieving the ACT engine
                hi = (
                    xt[:, 2 * q : 2 * q + 2]
                    .bitcast(mybir.dt.uint8)
                    .rearrange("p k (n two) -> p k n two", two=2)[:, :, :, 1]
                )
                nc.vector.tensor_scalar(
                    out=t1[:, 2 * q : 2 * q + 2].bitcast(mybir.dt.uint8),
                    in0=hi,
                    scalar1=0x80,
                    scalar2=0x38,
                    op0=mybir.AluOpType.bitwise_and,
                    op1=mybir.AluOpType.bitwise_or,
                )

            def sign_x(i):
                xt = xts[i]
                t1 = t1p.tile([128, 8, HW], F8, tag="t1")
                for q in range(4):
                    if q == 1:
                        sign_chunk_gpsimd(t1, xt, q)
                    else:
                        nc.scalar.activation(
                            out=t1[:, 2 * q : 2 * q + 2],
                            in_=xt[:, 2 * q : 2 * q + 2],
                            func=AF.Sign,
                        )
                t1s.append(t1)

            def conv1_img0():
                # image 0 only: kk-outer so each x-chunk is consumed by all 4
                # matmuls (both m accumulators) as soon as it is signed -- the
                # PE then keeps pace with the DMA arrival instead of waiting
                # for the full image.  The second accumulator borrows a c3p
                # slot (conv3 is not running yet).
                t1 = t1s[0]
                z1 = z1bufs[0]
                c1a = c1p.tile([128, 1024], F32, tag="c1")
                c1b = c3p.tile([128, 1024], F32, tag="c3")
                cs = (c1a, c1b)
                for kk in range(4):
                    for m in range(2):
                        for f in range(2):
                            nc.tensor.matmul(
                                out=cs[m][:, PSOFF[f] : PSOFF[f] + 392],
                                lhsT=w1s[:, 2 * kk : 2 * kk + 2, m * 128 : (m + 1) * 128],
                                rhs=t1[:, 2 * kk : 2 * kk + 2, f * 392 : (f + 1) * 392],
                                start=(kk == 0),
                                stop=(kk == 3),
                                perf_mode=DR,
                            )
                for m in range(2):
                    z1v = z1[:, m].rearrange("p (h w) -> p h w", w=PW)
                    nc.scalar.activation(
                        out=z1v[:, 1 : H + 1, 1 : W + 1],
                        in_=cs[m][:].rearrange("p (c n) -> p c n", n=512)[:, :, 0:392],
                        func=AF.Sign,
                        bias=b1s[:, m : m + 1],
                        scale=1.0,
                    )

            def conv1(i):
                # 1x1, 1024 -> 256; threshold sign into padded z1 (pad ring
                # stays zero from the one-time memset; sign writes interior)
                t1 = t1s[i]
                z1 = z1bufs[i % 2]
                for m in range(2):
                    c1 = c1p.tile([128, 1024], F32, tag="c1")
                    for kk in range(4):
                        for f in range(2):
                            nc.tensor.matmul(
                                out=c1[:, PSOFF[f] : PSOFF[f] + 392],
                                lhsT=w1s[:, 2 * kk : 2 * kk + 2, m * 128 : (m + 1) * 128],
                                rhs=t1[:, 2 * kk : 2 * kk + 2, f * 392 : (f + 1) * 392],
                                start=(kk == 0),
                                stop=(kk == 3),
                                perf_mode=DR,
                            )
                    z1v = z1[:, m].rearrange("p (h w) -> p h w", w=PW)
                    nc.scalar.activation(
                        out=z1v[:, 1 : H + 1, 1 : W + 1],
                        in_=c1[:].rearrange("p (c n) -> p c n", n=512)[:, :, 0:392],
                        func=AF.Sign,
                        bias=b1s[:, m : m + 1],
                        scale=1.0,
                    )

            c2live = {}

            def conv2_half(i, m, tlo, thi):
                # 3x3, pad 1, 256 -> 256 as 9 shifted accumulating matmuls
                # over the padded flat layout (pad-garbage columns computed
                # and discarded); threshold sign into compact z2.  Emitted in
                # tap-ranges so conv3 batches of the previous image can be
                # woven between them.
                if m == 0 and tlo == 0:
                    z2t = z2p.tile([128, 2, HW], F8, tag="z2")
                    z2s.append(z2t)
                z1, z2 = z1bufs[i % 2], z2s[i]
                if tlo == 0:
                    c2 = c2p.tile([128, 1024], F32, tag="c2")
                    c2live[(i, m)] = c2
                c2 = c2live[(i, m)]
                for t9 in range(tlo, thi):
                    dy, dx = t9 // 3, t9 % 3
                    for f, (base, nw) in enumerate(CHUNKS):
                        off = base + (dy - 1) * PW + (dx - 1)
                        nc.tensor.matmul(
                            out=c2[:, PSOFF[f] : PSOFF[f] + nw],
                            lhsT=w2s[:, :, t9, m * 128 : (m + 1) * 128],
                            rhs=z1[:, :, off : off + nw],
                            start=(t9 == 0),
                            stop=(t9 == 8),
                            perf_mode=DR,
                        )
                if thi == 9:
                    nc.scalar.activation(
                        out=z2[:, m],
                        in_=c2[:]
                        .rearrange("p (c n) -> p c n", c=2)[:, :, 0:420]
                        .rearrange("p c (h w) -> p c h w", w=PW)[:, :, :, :W],
                        func=AF.Sign,
                        bias=b2s[:, m : m + 1],
                        scale=1.0,
                    )

            def conv2_m(i, m):
                conv2_half(i, m, 0, 9)

            def conv3_part(i, m3lo, m3hi):
                # 1x1, 256 -> 1024; fused bn + residual + hardtanh; store
                if m3lo == 0:
                    ott = op.tile([128, 8, HW], BF16, tag="ot")
                    ots.append(ott)
                xt, z2, ot = xts[i], z2s[i], ots[i]
                for m3 in range(m3lo, m3hi):
                    c3 = c3p.tile([128, 1024], F32, tag="c3")
                    for f in range(2):
                        nc.tensor.matmul(
                            out=c3[:, PSOFF[f] : PSOFF[f] + 392],
                            lhsT=w3s[:, :, m3 * 128 : (m3 + 1) * 128],
                            rhs=z2[:, :, f * 392 : (f + 1) * 392],
                            start=True,
                            stop=True,
                            perf_mode=DR,
                        )
                    nc.vector._custom_dve(
                        caac,
                        out=ot[:, m3],
                        in0=c3[:].rearrange("p (c n) -> p c n", n=512)[:, :, 0:392],
                        in1=xt[:, m3],
                        s0=k3s[:, m3 : m3 + 1],
                        s1=b3s[:, m3 : m3 + 1],
                    )
                    if m3 in (3, 5, 7):
                        klo, khi = {3: (0, 4), 5: (4, 6), 7: (6, 8)}[m3]
                        nc.sync.dma_start(
                            out=o_d[:, i, klo:khi],
                            in_=ot[:, klo:khi],
                        )

            def load_x0_signx0():
                # first image: fine-grained chunks alternating over both
                # HWDGE queues (SP + ACT) so triggers issue in parallel and
                # the PE can start as soon as the first k-pair is signed
                xt = xp.tile([128, 8, HW], BF16, tag="xt")
                t1 = t1p.tile([128, 8, HW], F8, tag="t1")
                for q in range(4):
                    nc.sync.dma_start(
                        out=xt[:, 2 * q : 2 * q + 2],
                        in_=x_d[:, 0, 2 * q : 2 * q + 2],
                    )
                    if q == 0:
                        nc.sync.dma_start(out=w1s[:], in_=w1_d[:])
                        nc.sync.dma_start(out=b1s[:], in_=b1_d[:])
                for q in range(4):
                    if q % 2:
                        sign_chunk_gpsimd(t1, xt, q)
                    else:
                        nc.scalar.activation(
                            out=t1[:, 2 * q : 2 * q + 2],
                            in_=xt[:, 2 * q : 2 * q + 2],
                            func=AF.Sign,
                        )
                xts.append(xt)
                t1s.append(t1)

            ots = []
            # one-time zero of the two z1 pad buffers (pad ring is never
            # written afterwards; interior is fully overwritten per image)
            z1buf_a = z1p.tile([128, 2, PN], F8, tag="z1a")
            z1buf_b = z1p.tile([128, 2, PN], F8, tag="z1b")
            z1bufs = (z1buf_a, z1buf_b)
            nc.gpsimd.memset(z1bufs[0][:], 0.0)
            nc.gpsimd.memset(z1bufs[1][:], 0.0)
            load_x0_signx0()
            conv1_img0()
            nc.sync.dma_start(out=w2s[:], in_=w2_d[:])
            nc.sync.dma_start(out=b2s[:], in_=b2_d[:])
            load_x(1)
            conv2_m(0, 0)
            sign_x(1)
            nc.sync.dma_start(out=w3s[:], in_=w3_d[:])
            nc.sync.dma_start(out=k3s[:], in_=k3_d[:])
            nc.sync.dma_start(out=b3s[:], in_=b3_d[:])
            load_x(2)
            load_x(3)
            conv2_m(0, 1)
            # steady state: conv3(i) is woven between conv1/conv2 pieces of
            # image i+1 so the PE never idles behind the DVE-paced tail
            for i in range(NIMG):
                if i + 1 < NIMG:
                    conv1(i + 1)
                    conv3_part(i, 0, 2)
                    conv2_half(i + 1, 0, 0, 5)
                    conv3_part(i, 2, 4)
                    conv2_half(i + 1, 0, 5, 9)
                    if i + 2 < NIMG:
                        sign_x(i + 2)
                    conv3_part(i, 4, 6)
                    conv2_half(i + 1, 1, 0, 5)
                    conv3_part(i, 6, 8)
                    conv2_half(i + 1, 1, 5, 9)
                else:
                    conv3_part(i, 0, 8)
    _dedup_ldweights(nc)
    nc.compile()
    return nc


_NC_CACHE = []


def _get_nc():
    if not _NC_CACHE:
        _NC_CACHE.append(_build_nc())
    return _NC_CACHE[0]


def _sign_w(w):
    """sign(w - per-out-filter mean), matching the reference's f32 math."""
    try:
        import jax
        import jax.numpy as jnp

        cpu = jax.devices("cpu")[0]
        with jax.default_device(cpu):
            wj = jnp.asarray(w, dtype=jnp.float32)
            m = jnp.mean(wj, axis=(1, 2, 3), keepdims=True)
            return np.asarray(jnp.sign(wj - m))
    except Exception:
        w = w.astype(np.float32)
        m = np.mean(w, axis=(1, 2, 3), keepdims=True, dtype=np.float32)
        return np.sign(w - m).astype(np.float32)


def _thresholds(alpha, g, b, cmax, step):
    """Per-channel integer threshold T: sign(((c*alpha)*s)+b) > 0  <=>  c >= T,
    evaluated with the reference's exact f32 elementwise ops on every
    achievable integer conv output c in [-cmax, cmax] (stride `step`).
    Returns bias = -(T - step/2) so that Sign(c + bias) reproduces the sign."""
    f32 = np.float32
    s = (g.astype(f32) / np.sqrt(f32(1.0 + EPS))).astype(f32)
    cs = np.arange(-cmax, cmax + 1, step, dtype=f32)
    v = (cs[None, :] * alpha.astype(f32)[:, None]).astype(f32)
    v = (v * s[:, None]).astype(f32)
    v = (v + b.astype(f32)[:, None]).astype(f32)
    pos = v > 0
    # monotone in c (all multipliers >= 0); find first positive
    idx = np.argmax(pos, axis=1)
    has_pos = pos.any(axis=1)
    T = np.where(has_pos, -f32(cmax) + idx.astype(f32) * f32(step), f32(cmax + 2 * step))
    bias = -(T - f32(step) / f32(2.0))
    return bias.astype(f32)


def _prep_inputs(x, w1, a1, g1, b1, w2, a2, g2, b2, w3, a3, g3, b3):
    f32 = np.float32
    bw1 = _sign_w(w1)[:, :, 0, 0]            # [256, 1024]
    bw2 = _sign_w(w2)                        # [256, 256, 3, 3]
    bw3 = _sign_w(w3)[:, :, 0, 0]            # [1024, 256]

    w1t = np.ascontiguousarray(
        bw1.T.reshape(8, 128, 256).transpose(1, 0, 2)
    ).astype(NP_F8)
    w2t = np.ascontiguousarray(
        bw2.transpose(1, 2, 3, 0).reshape(2, 128, 9, 256).transpose(1, 0, 2, 3)
    ).astype(NP_F8)
    w3t = np.ascontiguousarray(
        bw3.T.reshape(2, 128, 1024).transpose(1, 0, 2)
    ).astype(NP_F8)

    bias1 = _thresholds(a1, g1, b1, 1024, 2)     # conv1 sums are even
    bias2 = _thresholds(a2, g2, b2, 2304, 1)     # conv2 sums any parity
    b1t = np.ascontiguousarray(bias1.reshape(2, 128).T)
    b2t = np.ascontiguousarray(bias2.reshape(2, 128).T)

    s3 = (g3.astype(f32) / np.sqrt(f32(1.0 + EPS))).astype(f32)
    k3 = (a3.astype(f32) * s3).astype(f32)
    k3t = np.ascontiguousarray(k3.reshape(8, 128).T)
    b3t = np.ascontiguousarray(b3.astype(f32).reshape(8, 128).T)

    shared = dict(w1t=w1t, w2t=w2t, w3t=w3t, b1t=b1t, b2t=b2t, k3t=k3t, b3t=b3t)
    in_maps = []
    # device layout is partition-major [128, NIMG, 8k, HW] so every DMA line
    # is >=3KB contiguous per partition
    xb = x.astype(NP_BF16).reshape(N, 8, 128, HW)
    for c in range(NCORES):
        xs = np.ascontiguousarray(
            xb[c * NIMG : (c + 1) * NIMG].transpose(2, 0, 1, 3)
        )
        in_maps.append(dict(x=xs, **shared))
    return in_maps


def kernel(**inputs):
    in_maps = _prep_inputs(
        inputs["x"],
        inputs["w1"], inputs["a1"], inputs["g1"], inputs["b1"],
        inputs["w2"], inputs["a2"], inputs["g2"], inputs["b2"],
        inputs["w3"], inputs["a3"], inputs["g3"], inputs["b3"],
    )
    nc = _get_nc()
    trace = bool(int(os.environ.get("KERNEL_TRACE", "0")))
    res = run_bass_kernel_spmd(nc, in_maps, list(range(NCORES)), trace=trace)
    if trace:
        kernel.last_results = res
    out = np.concatenate(
        [
            r["out"]
            .transpose(1, 2, 0, 3)
            .astype(np.float32)
            .reshape(NIMG, CIN, H, W)
            for r in res.results
        ],
        axis=0,
    )
    return out



# revision 41
# speedup vs baseline: 1.0440x; 1.0440x over previous
"""Trainium2 Bass kernel for nn_Bottleneck_1w1a (binarized ResNet bottleneck).

Math notes (why this is exact):
  - The reference binarizes weights as sign(standardize(w)); standardization
    divides by a positive scalar and the tau-clip is identity (tau=1 -> +inf),
    so bw = sign(w - mean(w, per out-filter)).
  - Activations are binarized as sign(a), and hardtanh(v) has the sign of v,
    so the input to conv_{k+1} is sign(bn_k(conv_k * alpha_k)).  With
    k = alpha*gamma/sqrt(1+eps) >= 0, sign(c*k + b) for the *integer* conv
    output c is a threshold comparison c >= T.  T is precomputed on host by
    evaluating the reference's exact f32 expression on every achievable
    integer c, so the device does Sign(c + bias) with bias = -(T - step/2),
    which is exact in f32.
  - All matmul operands are exactly +-1 (or 0 from padding) which fp8e4m3
    represents exactly; PSUM accumulates in f32, so conv outputs are exact
    integers.  fp8 DoubleRow mode (2 contraction subtiles/instr) is lossless.
  - conv2 (3x3, pad 1) is done as 9 shifted accumulating matmuls over a
    zero-padded 30x30 per-image layout; the pad garbage columns are computed
    and discarded via strided access patterns.

Sharding: data-parallel over batch, 4 images per core, weights replicated.
"""

import os
import sys

import numpy as np

for _p in ("/opt/trn_rl_repo", "/root/.axon_site/_ro/pypackages"):
    if _p not in sys.path:
        sys.path.insert(0, _p)

from concourse import bacc, bass, mybir  # noqa: E402
from concourse.tile import TileContext  # noqa: E402
from concourse.bass_utils import run_bass_kernel_spmd  # noqa: E402

F8 = mybir.dt.float8e4
F32 = mybir.dt.float32
BF16 = mybir.dt.bfloat16
NP_F8 = mybir.dt.np(F8)
NP_BF16 = mybir.dt.np(BF16)
AF = mybir.ActivationFunctionType
DR = mybir.MatmulPerfMode.DoubleRow

N, CIN, H, W = 32, 1024, 28, 28
NCORES = 8
NIMG = N // NCORES          # 4 images per core
HW = H * W                  # 784
PW = W + 2                  # 30 padded width
PN = PW * (H + 2)           # 900 padded flat size
EPS = 1e-5

# psum column offset for each half-image chunk (each matmul's out must sit in
# one 2KB PSUM bank)
PSOFF = (0, 512)
# conv2 free-dim chunks over padded flat coords j (interior j in [31,869)):
# (start_j, width).  Chunk A covers rows 1..14, chunk B rows 15..28; widths
# chosen so every tap read j + (dy-1)*30 + (dx-1) stays inside [0, 900).
# The columns with j % 30 in {0, 29} are pad garbage, discarded by the
# strided interior view in the z2 activation.
CHUNKS = ((31, 420), (451, 418))


def _register_caac():
    """Register a fused DVE op: out = clip((in0*s0 + s1) + in1, -1, 1).
    This is the whole bn3 + residual + hardtanh tail in one DVE pass.
    Runtime registration via the documented per-NEFF DVE table mechanism;
    the pinned uop hashes are computed with the same lowering the golden
    test uses."""
    import concourse.dve_ops as DO
    from concourse.dve_spec import Spec, Src0, Src1, C0, C1, Zero, One, maxx, minn, lower
    from concourse.dve_uop import DveOpSpec

    name = "CLAMP_AFFINE_ADD_ANT"
    for op in DO.OPS:
        if op.name == name:
            return op

    def ref(in0, in1, s0, s1, imm2):
        p = in0.shape[0]
        a = in0.astype(np.float32).reshape(p, -1)
        b = np.asarray(in1, dtype=np.float32).reshape(p, -1)
        out = np.clip((a * s0 + s1) + b, -1.0, 1.0).astype(np.float32)
        return out

    spec = Spec(
        body=minn(maxx((Src0 * C0 + C1) + Src1, Zero - One), One),
        reference=ref,
    )
    shas = {
        v: DveOpSpec(name=name, opcode=0, uops=lower(spec, ver=v), rd1_en=True).sha(v)
        for v in ("v3", "v4")
    }
    op = DO.DveOp(name, spec, subdim=False, uops_sha=shas)
    DO.OPS.append(op)
    DO._SUB_OPCODE_FOR_NAME[name] = DO._CUSTOM_DVE_ROW_BASE + len(DO.OPS) - 1
    DO.CUSTOM_DVE_SPECS[name] = spec
    return op


def _dedup_ldweights(nc):
    """Remove LDWEIGHTS instructions that reload the stationary weights the
    PE already holds (Tile lowering emits one per matmul; consecutive
    matmuls that share lhsT only need the first).  Between a kept load and
    a dependent matmul only Matmult instructions occur, which do not
    disturb the stationary array.  Any sync info on a removed load is
    transplanted to the next PE instruction so semaphore counts are
    preserved."""
    import json

    def ldw_key(inst):
        j = json.loads(mybir.instruction_to_pretty_json_string(inst))
        return json.dumps(
            (j.get("ins"), j.get("perf_mode"), j.get("is_transpose")), sort_keys=True
        )

    removed = 0
    for blk in nc.m.functions[0].blocks:
        insts = blk.instructions
        last_key = None
        to_remove = []
        pending_sync = None
        for inst in insts:
            if inst.engine != mybir.EngineType.PE:
                continue
            if pending_sync is not None:
                si = inst.sync_info
                if si is None:
                    inst.sync_info = pending_sync
                else:
                    si.on_wait.extend(pending_sync.on_wait)
                    si.on_update.extend(pending_sync.on_update)
                pending_sync = None
            if inst.opcode == "Ldweights":
                k = ldw_key(inst)
                if k == last_key:
                    if inst.sync_info is not None and (
                        inst.sync_info.on_wait or inst.sync_info.on_update
                    ):
                        pending_sync = inst.sync_info
                    to_remove.append(inst)
                last_key = k
            elif inst.opcode == "Matmult":
                pass
            else:
                last_key = None
        assert pending_sync is None
        for inst in to_remove:
            insts.remove(inst)
            removed += 1
    return removed


def _build_nc():
    caac = _register_caac()
    nc = bacc.Bacc()
    # all DMAs go through the SP HWDGE queue; dropping the unused Activation
    # HWDGE queue saves its 16 ring-drain markers in the NEFF teardown
    nc.m.queues = [q for q in nc.m.queues if q.name != "qActDynamicHW"]
    nc.hwdge_engines = type(nc.hwdge_engines)([mybir.EngineType.SP])
    x_d = nc.declare_dram_parameter("x", [128, NIMG, 8, HW], BF16, isOutput=False)
    w1_d = nc.declare_dram_parameter("w1t", [128, 8, 256], F8, isOutput=False)
    w2_d = nc.declare_dram_parameter("w2t", [128, 2, 9, 256], F8, isOutput=False)
    w3_d = nc.declare_dram_parameter("w3t", [128, 2, 1024], F8, isOutput=False)
    b1_d = nc.declare_dram_parameter("b1t", [128, 2], F32, isOutput=False)
    b2_d = nc.declare_dram_parameter("b2t", [128, 2], F32, isOutput=False)
    k3_d = nc.declare_dram_parameter("k3t", [128, 8], F32, isOutput=False)
    b3_d = nc.declare_dram_parameter("b3t", [128, 8], F32, isOutput=False)
    o_d = nc.declare_dram_parameter("out", [128, NIMG, 8, HW], BF16, isOutput=True)

    with TileContext(nc) as tc:
        with (
            tc.tile_pool(name="const", bufs=1) as cp,
            tc.tile_pool(name="xp", bufs=4) as xp,
            tc.tile_pool(name="t1p", bufs=2) as t1p,
            tc.tile_pool(name="z1p", bufs=2) as z1p,
            tc.tile_pool(name="z2p", bufs=2) as z2p,
            tc.tile_pool(name="op", bufs=2) as op,
            tc.tile_pool(name="c1p", bufs=1, space="PSUM") as c1p,
            tc.tile_pool(name="c2p", bufs=1, space="PSUM") as c2p,
            tc.tile_pool(name="c3p", bufs=2, space="PSUM") as c3p,
        ):
            w1s = cp.tile([128, 8, 256], F8)
            w2s = cp.tile([128, 2, 9, 256], F8)
            w3s = cp.tile([128, 2, 1024], F8)
            b1s = cp.tile([128, 2], F32)
            b2s = cp.tile([128, 2], F32)
            k3s = cp.tile([128, 8], F32)
            b3s = cp.tile([128, 8], F32)
            # Hand-pipelined schedule.  Stage order per iteration i:
            #   sign(i+1)  [DVE+Pool]  -> conv2(i) [PE] -> load_x(i+2) [SP]
            #   -> conv1(i+1) [PE, fills the z2(i) ACT gap]
            #   -> conv3(i)+tail(i) [PE/DVE/SP, fills the z1(i+1) ACT gap]
            xts, t1s, z1s, z2s = [], [], [], []

            def load_x(i):
                xt = xp.tile([128, 8, HW], BF16, tag="xt")
                for half in range(2):
                    nc.sync.dma_start(
                        out=xt[:, 4 * half : 4 * half + 4],
                        in_=x_d[:, i, 4 * half : 4 * half + 4],
                    )
                xts.append(xt)

            def sign_chunk_gpsimd(t1, xt, q):
                # sign(x) in fp8e4m3 is just (high_byte & 0x80) | 0x38, so
                # gpsimd can binarize from the bf16 sign bit with one
                # tensor_scalar, relieving the ACT engine
                hi = (
                    xt[:, 2 * q : 2 * q + 2]
                    .bitcast(mybir.dt.uint8)
                    .rearrange("p k (n two) -> p k n two", two=2)[:, :, :, 1]
                )
                nc.vector.tensor_scalar(
                    out=t1[:, 2 * q : 2 * q + 2].bitcast(mybir.dt.uint8),
                    in0=hi,
                    scalar1=0x80,
                    scalar2=0x38,
                    op0=mybir.AluOpType.bitwise_and,
                    op1=mybir.AluOpType.bitwise_or,
                )

            def sign_x(i):
                xt = xts[i]
                t1 = t1p.tile([128, 8, HW], F8, tag="t1")
                for q in range(4):
                    if q == 1:
                        sign_chunk_gpsimd(t1, xt, q)
                    else:
                        nc.scalar.activation(
                            out=t1[:, 2 * q : 2 * q + 2],
                            in_=xt[:, 2 * q : 2 * q + 2],
                            func=AF.Sign,
                        )
                t1s.append(t1)

            def conv1_img0():
                # image 0 only: kk-outer so each x-chunk is consumed by all 4
                # matmuls (both m accumulators) as soon as it is signed -- the
                # PE then keeps pace with the DMA arrival instead of waiting
                # for the full image.  The second accumulator borrows a c3p
                # slot (conv3 is not running yet).
                t1 = t1s[0]
                z1 = z1bufs[0]
                c1a = c1p.tile([128, 1024], F32, tag="c1")
                c1b = c3p.tile([128, 1024], F32, tag="c3")
                cs = (c1a, c1b)
                for kk in range(4):
                    for m in range(2):
                        for f in range(2):
                            nc.tensor.matmul(
                                out=cs[m][:, PSOFF[f] : PSOFF[f] + 392],
                                lhsT=w1s[:, 2 * kk : 2 * kk + 2, m * 128 : (m + 1) * 128],
                                rhs=t1[:, 2 * kk : 2 * kk + 2, f * 392 : (f + 1) * 392],
                                start=(kk == 0),
                                stop=(kk == 3),
                                perf_mode=DR,
                            )
                for m in range(2):
                    z1v = z1[:, m].rearrange("p (h w) -> p h w", w=PW)
                    nc.scalar.activation(
                        out=z1v[:, 1 : H + 1, 1 : W + 1],
                        in_=cs[m][:].rearrange("p (c n) -> p c n", n=512)[:, :, 0:392],
                        func=AF.Sign,
                        bias=b1s[:, m : m + 1],
                        scale=1.0,
                    )

            def conv1(i):
                # 1x1, 1024 -> 256; threshold sign into padded z1 (pad ring
                # stays zero from the one-time memset; sign writes interior)
                t1 = t1s[i]
                z1 = z1bufs[i % 2]
                for m in range(2):
                    c1 = c1p.tile([128, 1024], F32, tag="c1")
                    for kk in range(4):
                        for f in range(2):
                            nc.tensor.matmul(
                                out=c1[:, PSOFF[f] : PSOFF[f] + 392],
                                lhsT=w1s[:, 2 * kk : 2 * kk + 2, m * 128 : (m + 1) * 128],
                                rhs=t1[:, 2 * kk : 2 * kk + 2, f * 392 : (f + 1) * 392],
                                start=(kk == 0),
                                stop=(kk == 3),
                                perf_mode=DR,
                            )
                    z1v = z1[:, m].rearrange("p (h w) -> p h w", w=PW)
                    nc.scalar.activation(
                        out=z1v[:, 1 : H + 1, 1 : W + 1],
                        in_=c1[:].rearrange("p (c n) -> p c n", n=512)[:, :, 0:392],
                        func=AF.Sign,
                        bias=b1s[:, m : m + 1],
                        scale=1.0,
                    )

            c2live = {}

            def conv2_half(i, m, tlo, thi):
                # 3x3, pad 1, 256 -> 256 as 9 shifted accumulating matmuls
                # over the padded flat layout (pad-garbage columns computed
                # and discarded); threshold sign into compact z2.  Emitted in
                # tap-ranges so conv3 batches of the previous image can be
                # woven between them.
                if m == 0 and tlo == 0:
                    z2t = z2p.tile([128, 2, HW], F8, tag="z2")
                    z2s.append(z2t)
                z1, z2 = z1bufs[i % 2], z2s[i]
                if tlo == 0:
                    c2 = c2p.tile([128, 1024], F32, tag="c2")
                    c2live[(i, m)] = c2
                c2 = c2live[(i, m)]
                for t9 in range(tlo, thi):
                    dy, dx = t9 // 3, t9 % 3
                    for f, (base, nw) in enumerate(CHUNKS):
                        off = base + (dy - 1) * PW + (dx - 1)
                        nc.tensor.matmul(
                            out=c2[:, PSOFF[f] : PSOFF[f] + nw],
                            lhsT=w2s[:, :, t9, m * 128 : (m + 1) * 128],
                            rhs=z1[:, :, off : off + nw],
                            start=(t9 == 0),
                            stop=(t9 == 8),
                            perf_mode=DR,
                        )
                if thi == 9:
                    nc.scalar.activation(
                        out=z2[:, m],
                        in_=c2[:]
                        .rearrange("p (c n) -> p c n", c=2)[:, :, 0:420]
                        .rearrange("p c (h w) -> p c h w", w=PW)[:, :, :, :W],
                        func=AF.Sign,
                        bias=b2s[:, m : m + 1],
                        scale=1.0,
                    )

            def conv2_m(i, m):
                conv2_half(i, m, 0, 9)

            def conv3_part(i, m3lo, m3hi):
                # 1x1, 256 -> 1024; fused bn + residual + hardtanh; store
                if m3lo == 0:
                    ott = op.tile([128, 8, HW], BF16, tag="ot")
                    ots.append(ott)
                xt, z2, ot = xts[i], z2s[i], ots[i]
                for m3 in range(m3lo, m3hi):
                    c3 = c3p.tile([128, 1024], F32, tag="c3")
                    for f in range(2):
                        nc.tensor.matmul(
                            out=c3[:, PSOFF[f] : PSOFF[f] + 392],
                            lhsT=w3s[:, :, m3 * 128 : (m3 + 1) * 128],
                            rhs=z2[:, :, f * 392 : (f + 1) * 392],
                            start=True,
                            stop=True,
                            perf_mode=DR,
                        )
                    nc.vector._custom_dve(
                        caac,
                        out=ot[:, m3],
                        in0=c3[:].rearrange("p (c n) -> p c n", n=512)[:, :, 0:392],
                        in1=xt[:, m3],
                        s0=k3s[:, m3 : m3 + 1],
                        s1=b3s[:, m3 : m3 + 1],
                    )
                    if m3 in (3, 5, 7):
                        klo, khi = {3: (0, 4), 5: (4, 6), 7: (6, 8)}[m3]
                        nc.sync.dma_start(
                            out=o_d[:, i, klo:khi],
                            in_=ot[:, klo:khi],
                        )

            def load_x0_signx0():
                # first image: fine-grained chunks alternating over both
                # HWDGE queues (SP + ACT) so triggers issue in parallel and
                # the PE can start as soon as the first k-pair is signed
                xt = xp.tile([128, 8, HW], BF16, tag="xt")
                t1 = t1p.tile([128, 8, HW], F8, tag="t1")
                for q in range(4):
                    nc.sync.dma_start(
                        out=xt[:, 2 * q : 2 * q + 2],
                        in_=x_d[:, 0, 2 * q : 2 * q + 2],
                    )
                    if q == 0:
                        nc.sync.dma_start(out=w1s[:], in_=w1_d[:])
                        nc.sync.dma_start(out=b1s[:], in_=b1_d[:])
                for q in range(4):
                    if q % 2:
                        sign_chunk_gpsimd(t1, xt, q)
                    else:
                        nc.scalar.activation(
                            out=t1[:, 2 * q : 2 * q + 2],
                            in_=xt[:, 2 * q : 2 * q + 2],
                            func=AF.Sign,
                        )
                xts.append(xt)
                t1s.append(t1)

            ots = []
            # one-time zero of the two z1 pad buffers (pad ring is never
            # written afterwards; interior is fully overwritten per image)
            z1buf_a = z1p.tile([128, 2, PN], F8, tag="z1a")
            z1buf_b = z1p.tile([128, 2, PN], F8, tag="z1b")
            z1bufs = (z1buf_a, z1buf_b)
            nc.gpsimd.memset(z1bufs[0][:], 0.0)
            nc.gpsimd.memset(z1bufs[1][:], 0.0)
            load_x0_signx0()
            conv1_img0()
            nc.sync.dma_start(out=w2s[:], in_=w2_d[:])
            nc.sync.dma_start(out=b2s[:], in_=b2_d[:])
            load_x(1)
            conv2_m(0, 0)
            sign_x(1)
            nc.sync.dma_start(out=w3s[:], in_=w3_d[:])
            nc.sync.dma_start(out=k3s[:], in_=k3_d[:])
            nc.sync.dma_start(out=b3s[:], in_=b3_d[:])
            load_x(2)
            load_x(3)
            conv2_m(0, 1)
            # steady state: conv3(i) is woven between conv1/conv2 pieces of
            # image i+1 so the PE never idles behind the DVE-paced tail
            for i in range(NIMG):
                if i + 1 < NIMG:
                    conv1(i + 1)
                    conv3_part(i, 0, 2)
                    conv2_half(i + 1, 0, 0, 5)
                    conv3_part(i, 2, 4)
                    conv2_half(i + 1, 0, 5, 9)
                    if i + 2 < NIMG:
                        sign_x(i + 2)
                    conv3_part(i, 4, 6)
                    conv2_half(i + 1, 1, 0, 5)
                    conv3_part(i, 6, 8)
                    conv2_half(i + 1, 1, 5, 9)
                else:
                    conv3_part(i, 0, 8)
    _dedup_ldweights(nc)
    nc.compile()
    return nc


_NC_CACHE = []


def _get_nc():
    if not _NC_CACHE:
        _NC_CACHE.append(_build_nc())
    return _NC_CACHE[0]


def _sign_w(w):
    """sign(w - per-out-filter mean), matching the reference's f32 math."""
    try:
        import jax
        import jax.numpy as jnp

        cpu = jax.devices("cpu")[0]
        with jax.default_device(cpu):
            wj = jnp.asarray(w, dtype=jnp.float32)
            m = jnp.mean(wj, axis=(1, 2, 3), keepdims=True)
            return np.asarray(jnp.sign(wj - m))
    except Exception:
        w = w.astype(np.float32)
        m = np.mean(w, axis=(1, 2, 3), keepdims=True, dtype=np.float32)
        return np.sign(w - m).astype(np.float32)


def _thresholds(alpha, g, b, cmax, step):
    """Per-channel integer threshold T: sign(((c*alpha)*s)+b) > 0  <=>  c >= T,
    evaluated with the reference's exact f32 elementwise ops on every
    achievable integer conv output c in [-cmax, cmax] (stride `step`).
    Returns bias = -(T - step/2) so that Sign(c + bias) reproduces the sign."""
    f32 = np.float32
    s = (g.astype(f32) / np.sqrt(f32(1.0 + EPS))).astype(f32)
    cs = np.arange(-cmax, cmax + 1, step, dtype=f32)
    v = (cs[None, :] * alpha.astype(f32)[:, None]).astype(f32)
    v = (v * s[:, None]).astype(f32)
    v = (v + b.astype(f32)[:, None]).astype(f32)
    pos = v > 0
    # monotone in c (all multipliers >= 0); find first positive
    idx = np.argmax(pos, axis=1)
    has_pos = pos.any(axis=1)
    T = np.where(has_pos, -f32(cmax) + idx.astype(f32) * f32(step), f32(cmax + 2 * step))
    bias = -(T - f32(step) / f32(2.0))
    return bias.astype(f32)


def _prep_inputs(x, w1, a1, g1, b1, w2, a2, g2, b2, w3, a3, g3, b3):
    f32 = np.float32
    bw1 = _sign_w(w1)[:, :, 0, 0]            # [256, 1024]
    bw2 = _sign_w(w2)                        # [256, 256, 3, 3]
    bw3 = _sign_w(w3)[:, :, 0, 0]            # [1024, 256]

    w1t = np.ascontiguousarray(
        bw1.T.reshape(8, 128, 256).transpose(1, 0, 2)
    ).astype(NP_F8)
    w2t = np.ascontiguousarray(
        bw2.transpose(1, 2, 3, 0).reshape(2, 128, 9, 256).transpose(1, 0, 2, 3)
    ).astype(NP_F8)
    w3t = np.ascontiguousarray(
        bw3.T.reshape(2, 128, 1024).transpose(1, 0, 2)
    ).astype(NP_F8)

    bias1 = _thresholds(a1, g1, b1, 1024, 2)     # conv1 sums are even
    bias2 = _thresholds(a2, g2, b2, 2304, 1)     # conv2 sums any parity
    b1t = np.ascontiguousarray(bias1.reshape(2, 128).T)
    b2t = np.ascontiguousarray(bias2.reshape(2, 128).T)

    s3 = (g3.astype(f32) / np.sqrt(f32(1.0 + EPS))).astype(f32)
    k3 = (a3.astype(f32) * s3).astype(f32)
    k3t = np.ascontiguousarray(k3.reshape(8, 128).T)
    b3t = np.ascontiguousarray(b3.astype(f32).reshape(8, 128).T)

    shared = dict(w1t=w1t, w2t=w2t, w3t=w3t, b1t=b1t, b2t=b2t, k3t=k3t, b3t=b3t)
    in_maps = []
    # device layout is partition-major [128, NIMG, 8k, HW] so every DMA line
    # is >=3KB contiguous per partition
    xb = x.astype(NP_BF16).reshape(N, 8, 128, HW)
    for c in range(NCORES):
        xs = np.ascontiguousarray(
            xb[c * NIMG : (c + 1) * NIMG].transpose(2, 0, 1, 3)
        )
        in_maps.append(dict(x=xs, **shared))
    return in_maps


def kernel(**inputs):
    in_maps = _prep_inputs(
        inputs["x"],
        inputs["w1"], inputs["a1"], inputs["g1"], inputs["b1"],
        inputs["w2"], inputs["a2"], inputs["g2"], inputs["b2"],
        inputs["w3"], inputs["a3"], inputs["g3"], inputs["b3"],
    )
    nc = _get_nc()
    trace = bool(int(os.environ.get("KERNEL_TRACE", "0")))
    res = run_bass_kernel_spmd(nc, in_maps, list(range(NCORES)), trace=trace)
    if trace:
        kernel.last_results = res
    out = np.concatenate(
        [
            r["out"]
            .transpose(1, 2, 0, 3)
            .astype(np.float32)
            .reshape(NIMG, CIN, H, W)
            for r in res.results
        ],
        axis=0,
    )
    return out

